# revision 12
# baseline (speedup 1.0000x reference)
"""Trainium2 Bass kernel for masked multi-head attention (B=2, S=2048, H=16, D=64).

Sharding: 8 cores = (2 batches) x (4 groups of 4 heads). Each core computes
qkv for its 4 heads + flash-style attention fully on-chip.

Host-side prep per core:
  - xT    [1024, 2048] = x[b].T, pre-rounded to float32r (11-bit mantissa RNE)
  - xkvT  [1024, n_kv] = x[b][mask==1].T (padded to mult of 128), pre-rounded
  - w     [1024, 768]  = W_qkv columns for this head group (q|k|v), pre-rounded
  - bias  [128, n_kv/128] = 0 for real keys, -1e9 for padding (exp -> 0)
Device returns outT [260, 2048] = 4 heads x (64 out rows + 1 softmax-sum row),
host divides and transposes into the final [2, 2048, 1024].
"""

import os
import sys

sys.path.insert(0, "/opt/trn_rl_repo")

import numpy as np

import concourse.bass as bass  # noqa: F401
import concourse.tile as tile
from concourse import bacc, mybir
from concourse.bass_utils import run_bass_kernel_spmd

B, S, DIM = 2, 2048, 1024
HEAD, HEAD_DIM = 16, 64
NEG = np.float32(-1e9)
NFI = DIM // 128  # 8 contraction tiles
F32 = mybir.dt.float32
F32R = mybir.dt.float32r

_CACHE = {}


def _round_f32r(x: np.ndarray) -> np.ndarray:
    """Round-to-nearest-even keeping 11 mantissa bits (hw float32r rounding)."""
    i = np.ascontiguousarray(x, dtype=np.float32).view(np.uint32).astype(np.uint64)
    shift = np.uint64(12)
    half = np.uint64(1 << 11)
    mask = np.uint64((1 << 12) - 1)
    r = (i + half - np.uint64(1) + ((i >> shift) & np.uint64(1))) & ~mask
    return r.astype(np.uint32).view(np.float32)


def _chunks(total, maxc=512):
    n = -(-total // maxc)
    base = -(-total // (n * 128)) * 128
    out = []
    off = 0
    while off < total:
        w = min(base, total - off)
        out.append((off, w))
        off += w
    return out


def _emit_body(nc, tc, pools, dram, n_kv, compressed):
    NKT = n_kv // 128
    big, ps, ps2, ptp, osp = pools
    xT_d, w_d, bias_d, outT_d = dram
    qchunks = _chunks(S)
    kchunks = _chunks(n_kv)
    assert all(wd == 512 for _, wd in qchunks)

    # preload the exp table while DMAs run
    warm = big.tile([128, 1], F32, tag="warm", name="warm")
    nc.gpsimd.memset(warm[:], 1.0)
    nc.scalar.activation(warm[:], warm[:], mybir.ActivationFunctionType.Exp)

    w_s = big.tile([128, NFI, 768], F32R, tag="w", name="w_s")
    w_ap = w_d.ap().rearrange("(a p) f -> p a f", p=128)
    bias_s = big.tile([128, NKT], F32, tag="bias", name="bias_s")
    xT_ap = xT_d.ap().rearrange("(a p) t -> p a t", p=128)
    xts = big.tile([128, NFI, S], F32R, tag="xts", name="xts")

    # DMA order = first-needed-first; chunk 0 stripped per-fi so the first
    # accumulations pipeline with arrival
    nc.sync.dma_start(w_s[:, :, 256:512], w_ap[:, :, 256:512])
    for fi in range(NFI):
        nc.sync.dma_start(xts[:, fi, 0:512], xT_ap[:, fi, 0:512])
    nc.sync.dma_start(bias_s[:], bias_d.ap())
    nc.sync.dma_start(w_s[:, :, 0:128], w_ap[:, :, 0:128])
    nc.sync.dma_start(w_s[:, :, 512:768], w_ap[:, :, 512:768])
    nc.sync.dma_start(
        xts[:, :, 512 : qchunks[1][0] + qchunks[1][1]],
        xT_ap[:, :, 512 : qchunks[1][0] + qchunks[1][1]],
    )
    nc.sync.dma_start(w_s[:, :, 128:256], w_ap[:, :, 128:256])
    for off, wd in qchunks[2:]:
        nc.sync.dma_start(xts[:, :, off : off + wd], xT_ap[:, :, off : off + wd])
    xkv = xts  # kept keys are the first n_kv (host-permuted) columns

    qT = [big.tile([128, S], F32R, tag=f"qT{p}", name=f"qT{p}") for p in range(2)]
    kT = [
        big.tile([128, n_kv], F32R, tag=f"kT{p}", name=f"kT{p}") for p in range(2)
    ]
    va = big.tile([128, NKT, 4, 65], F32R, tag="va", name="va")
    ones = big.tile([128, 4, 1], F32, tag="ones", name="ones")
    nc.gpsimd.memset(ones[:], 1.0)
    for jt in range(NKT):
        nc.vector.tensor_copy(va[:, jt, :, 64:65], ones[:])

    def emit_k_chunk(p, off, wd):
        acc = ps.tile([128, 1024], F32, tag="wide", name="acc_k")
        for sub in range(0, wd, 512):
            sw = min(512, wd - sub)
            for fi in range(NFI):
                nc.tensor.matmul(
                    acc[:, sub : sub + sw],
                    w_s[:, fi, 256 + 128 * p : 256 + 128 * (p + 1)],
                    xkv[:, fi, off + sub : off + sub + sw],
                    start=(fi == 0),
                    stop=(fi == NFI - 1),
                )
        nc.vector.tensor_copy(kT[p][:, off : off + wd], acc[:, :wd])

    def emit_k(p):
        for off, wd in kchunks:
            emit_k_chunk(p, off, wd)

    def emit_v_range(j0, j1):
        for jt in range(j0, j1):
            acc = ps.tile([128, 1024], F32, tag="wide", name="acc_v")
            for fi in range(NFI):
                nc.tensor.matmul(
                    acc[:, :256],
                    xkv[:, fi, jt * 128 : (jt + 1) * 128],
                    w_s[:, fi, 512:768],
                    start=(fi == 0),
                    stop=(fi == NFI - 1),
                )
            nc.vector.tensor_copy(
                va[:, jt, :, 0:64],
                acc[:, :256].rearrange("p (h d) -> p h d", h=4),
            )

    def emit_q(p, ci):
        off, wd = qchunks[ci]
        acc = ps.tile([128, 1024], F32, tag="wide", name="acc_q")
        for fi in range(NFI):
            nc.tensor.matmul(
                acc[:, :wd],
                w_s[:, fi, 128 * p : 128 * (p + 1)],
                xts[:, fi, off : off + wd],
                start=(fi == 0),
                stop=(fi == NFI - 1),
            )
        nc.vector.tensor_copy(qT[p][:, off : off + wd], acc[:, :wd])

    # emission ordered by DMA arrival: work gated on x chunk i comes before
    # anything gated on chunk i+1; later k chunks + v tiles fold into block 0
    emit_k_chunk(0, *kchunks[0])
    emit_v_range(0, (kchunks[0][0] + kchunks[0][1]) // 128)
    emit_q(0, 0)
    emit_q(0, 1)

    inner0 = {}
    for off, wd in kchunks[1:]:
        inner0.setdefault(off // 128, []).append(
            lambda off=off, wd=wd: (
                emit_k_chunk(0, off, wd),
                emit_v_range(off // 128, (off + wd) // 128),
            )
        )

    # fine-grained filler pieces: (needed_by_block, callable); one matmul each
    queue = []

    def q_pieces(p, ci, needed_by):
        off, wd = qchunks[ci]
        cell = []

        def mk(fi):
            def f():
                if not cell:
                    cell.append(
                        ps.tile([128, 1024], F32, tag="wide", name="acc_qf")
                    )
                nc.tensor.matmul(
                    cell[0][:, :wd],
                    w_s[:, fi, 128 * p : 128 * (p + 1)],
                    xts[:, fi, off : off + wd],
                    start=(fi == 0),
                    stop=(fi == NFI - 1),
                )

            return f

        for fi in range(NFI):
            queue.append((needed_by, mk(fi)))
        queue.append(
            (
                needed_by,
                lambda: nc.vector.tensor_copy(
                    qT[p][:, off : off + wd], cell[0][:, :wd]
                ),
            )
        )

    def k_pieces(p, off, wd, needed_by):
        cell = []

        def mk(sub, sw, fi):
            def f():
                if not cell:
                    cell.append(
                        ps.tile([128, 1024], F32, tag="wide", name="acc_kf")
                    )
                nc.tensor.matmul(
                    cell[0][:, sub : sub + sw],
                    w_s[:, fi, 256 + 128 * p : 256 + 128 * (p + 1)],
                    xkv[:, fi, off + sub : off + sub + sw],
                    start=(fi == 0),
                    stop=(fi == NFI - 1),
                )

            return f

        for sub in range(0, wd, 512):
            sw = min(512, wd - sub)
            for fi in range(NFI):
                queue.append((needed_by, mk(sub, sw, fi)))
        queue.append(
            (
                needed_by,
                lambda: nc.vector.tensor_copy(
                    kT[p][:, off : off + wd], cell[0][:, :wd]
                ),
            )
        )

    nq = len(qchunks)
    q_pieces(0, 2, needed_by=2)
    q_pieces(0, 3, needed_by=3)
    for off, wd in kchunks:
        k_pieces(1, off, wd, needed_by=nq)
    for ci in range(nq):
        q_pieces(1, ci, needed_by=nq + ci)

    # ---- attention: pair p covers local heads 2p (A) and 2p+1 (B) ----
    Exp = mybir.ActivationFunctionType.Exp
    blocks = [(p, ci) for p in range(2) for ci in range(len(qchunks))]

    def drain_required(bi):
        while queue and queue[0][0] <= bi:
            queue.pop(0)[1]()

    def pull(n):
        while n > 0 and queue:
            queue.pop(0)[1]()
            n -= 1

    for bi, (p, ci) in enumerate(blocks):
        drain_required(bi)
        coff, cw = qchunks[ci]
        pv = ps2.tile([65, 1024], F32, tag="pv", name="pv")
        inner = inner0 if bi == 0 else {}

        def scores(j):
            sc = ps.tile([128, 1024], F32, tag="wide", name="sc")
            for i in range(2):
                lo = 64 * i
                nc.tensor.matmul(
                    sc[:, 512 * i : 512 * i + cw],
                    kT[p][lo : lo + 64, j * 128 : (j + 1) * 128],
                    qT[p][lo : lo + 64, coff : coff + cw],
                    start=True,
                    stop=True,
                )
            pt = ptp.tile([128, 1024], F32R, tag="pt", name="pt")
            nc.scalar.activation(pt[:], sc[:], Exp, bias=bias_s[:, j : j + 1])
            return pt

        def pv_mm(j, pt):
            for i in range(2):
                nc.tensor.matmul(
                    pv[:, 512 * i : 512 * i + cw],
                    va[:, j, 2 * p + i, :],
                    pt[:, 512 * i : 512 * i + cw],
                    start=(j == 0),
                    stop=(j == NKT - 1),
                )

        prev = scores(0)
        for j in range(1, NKT):
            for f in inner.get(j, []):
                f()
            cur = scores(j)
            if not inner:
                pull(2)
            pv_mm(j - 1, prev)
            prev = cur
        pv_mm(NKT - 1, prev)

        for i in range(2):
            o = osp.tile([65, 512], F32, tag="o", name="o")
            nc.vector.tensor_copy(o[:, :cw], pv[:, 512 * i : 512 * i + cw])
            lh = 2 * p + i
            nc.sync.dma_start(
                outT_d.ap()[65 * lh : 65 * lh + 65, coff : coff + cw],
                o[:, :cw],
            )


def _build(n_kv: int, compressed: bool, reps: int = 1):
    """Build the per-core Bass graph. Same graph runs SPMD on all 8 cores."""
    nc = bacc.Bacc("TRN2", target_bir_lowering=False, debug=False)

    NKT = n_kv // 128
    xT_d = nc.dram_tensor("xT", [DIM, S], F32R, kind="ExternalInput")
    w_d = nc.dram_tensor("w", [DIM, 768], F32R, kind="ExternalInput")
    bias_d = nc.dram_tensor("bias", [128, NKT], F32, kind="ExternalInput")
    outT_d = nc.dram_tensor("outT", [260, S], F32, kind="ExternalOutput")
    dram = (xT_d, w_d, bias_d, outT_d)

    with tile.TileContext(nc) as tc:
        with (
            tc.tile_pool(name="big", bufs=1) as big,
            tc.tile_pool(name="ps", bufs=3, space="PSUM") as ps,
            tc.tile_pool(name="ps2", bufs=1, space="PSUM") as ps2,
            tc.tile_pool(name="ptp", bufs=4) as ptp,
            tc.tile_pool(name="osp", bufs=4) as osp,
        ):
            pools = (big, ps, ps2, ptp, osp)
            for rep in range(reps):
                if rep:
                    tc.strict_bb_all_engine_barrier()
                _emit_body(nc, tc, pools, dram, n_kv, compressed)

    nc.compile()
    return nc


def _get_graph(n_kv: int, compressed: bool, reps: int = 1):
    key = (n_kv, compressed, reps)
    if key not in _CACHE:
        _CACHE[key] = _build(n_kv, compressed, reps)
    return _CACHE[key]


def prepare(x, W_qkv, mask, reps: int = 1):
    """Host-side prep: returns (nc, in_maps)."""
    x = np.asarray(x, dtype=np.float32)
    W_qkv = np.asarray(W_qkv, dtype=np.float32)
    mask = np.asarray(mask)

    keep = [np.nonzero(mask[b] != 0)[0] for b in range(B)]
    n_keep = max(len(k) for k in keep)
    n_kv = min(S, max(128, -(-n_keep // 128) * 128))
    compressed = True

    # permute tokens: kept (unmasked) first, rest after; k/v use first n_kv
    perms, xT, biases = [], [], []
    for b in range(B):
        unkept = np.nonzero(mask[b] == 0)[0]
        perm = np.concatenate([keep[b], unkept])
        perms.append(perm)
        xT.append(_round_f32r(x[b][perm].T))
        bv = np.full(n_kv, NEG, np.float32)
        bv[: len(keep[b])] = 0.0
        biases.append(np.ascontiguousarray(bv.reshape(-1, 128).T))

    wg = []
    for g in range(4):
        cols = np.concatenate(
            [
                W_qkv[:, 256 * g : 256 * (g + 1)],
                W_qkv[:, 1024 + 256 * g : 1024 + 256 * (g + 1)],
                W_qkv[:, 2048 + 256 * g : 2048 + 256 * (g + 1)],
            ],
            axis=1,
        )
        wg.append(_round_f32r(cols))

    in_maps = []
    for c in range(8):
        b, g = c // 4, c % 4
        in_maps.append({"xT": xT[b], "w": wg[g], "bias": biases[b]})

    nc = _get_graph(n_kv, compressed, reps)
    return nc, in_maps, perms


def assemble(results, perms):
    out = np.empty((B, S, DIM), np.float32)
    for c in range(8):
        b, g = c // 4, c % 4
        outT = results[c]["outT"]
        for i in range(4):
            h = 4 * g + i
            rows = outT[65 * i : 65 * i + 64]
            sums = outT[65 * i + 64]
            out[b, perms[b], 64 * h : 64 * (h + 1)] = (rows / sums).T
    return out


def run(x, W_qkv, mask, trace=False, tmpdir=None):
    nc, in_maps, perms = prepare(x, W_qkv, mask)
    res = run_bass_kernel_spmd(
        nc, in_maps, core_ids=list(range(8)), trace=trace, tmpdir=tmpdir
    )
    return assemble(res.results, perms), res


def kernel(x, W_qkv, mask):
    out, _ = run(x, W_qkv, mask, trace=os.environ.get("KERNEL_TRACE") == "1")
    return out
